# revision 3
# baseline (speedup 1.0000x reference)
"""Trainium2 Bass kernel for the e3nn-style 5x5x5 SAME conv (dense_cnn).

Strategy
--------
Data-parallel: 8 shards = 2 batches x 4 x-slabs of 12 output planes each.
Each core gets a zero/halo-padded, channel-first input slab [64, 16, 52, 52]
and produces [64, 12, 48, 48].

On-device the conv is a sum of 75 (=5x5x3) PSUM-accumulated matmuls per
output tile:
  - SBUF holds each input x-plane as a "dup" tile [128, 52*52]:
    partitions 0-63  = channel c at voxel v
    partitions 64-127= channel c at voxel v+1 (one z-voxel shift)
  - A matmul with K=128 therefore applies TWO z-taps at once, and the
    stationary weights [128, 128] map them to TWO output z-parities
    (out partitions = (parity p, channel o)). Covering tz in 0..4 takes
    3 matmuls (z-bases 0, 2, 4) per (tx, ty): 10 of 12 weight blocks
    useful -> 83% PE utilization ceiling.
  - Weights/moving data are fp32 bitcast to float32r (full-rate fp32
    matmul for moving dim >= 256; slight HW precision reduction).

The tiny 5x5x5x64x64 kernel build (radial basis x Clebsch-Gordan) is done
on the host in numpy and shipped as a packed [128, 75*128] weight input,
replicated to every core.
"""

import math

import numpy as np

import concourse.bass as bass
import concourse.mybir as mybir
from concourse import bacc, bass_utils
from concourse.tile import TileContext

MUL = 16
NB = 4
R = 2.5

N_CORES = 8
PX, PY, PZ = 16, 52, 52          # padded per-core input slab (x, y, z)
OX, OY, OZ = 12, 48, 48          # per-core output region
PLANE = PY * PZ                  # 2704 voxels per x-plane
OPLANE = OY * OZ                 # 2304 outputs per x-plane
ZB_LIST = (0, 2, 4)              # z-base offsets; taps tz = zb + s - p
N_W = 5 * 5 * 3                  # 75 packed weight matrices
YB = 3                           # y-blocks of 16 rows -> N = 16*24 = 384
YBS = OY // YB


def _build_k(w000, w011, w101, w110, sc0, sc1):
    """Numpy port of the reference kernel build. Returns [5,5,5,64,64]."""
    s = 2
    c = np.arange(-s, s + 1.0)
    lat = np.stack(np.meshgrid(c, c, c, indexing='ij'), axis=-1)
    norm = np.linalg.norm(lat, axis=-1)
    safe = np.where(norm == 0.0, 1.0, norm)
    nvec = np.where(norm[..., None] > 0.0, lat / safe[..., None], 0.0)
    sh1 = np.sqrt(3.0) * nvec
    values = np.linspace(0.0, R, NB + 2)[1:-1]
    step = R / (NB + 1)
    d = (norm[..., None] - values) / step
    dd = np.clip(d, -1.0 + 1e-9, 1.0 - 1e-9)
    emb = np.where(np.abs(d) < 1.0,
                   1.14136 * np.e ** 2 * np.exp(-1.0 / (1.0 - dd ** 2)), 0.0)
    nlat = 125.0

    r000 = np.einsum('xyzb,buw->xyzuw', emb, w000) / nlat
    r011 = np.einsum('xyzb,buw->xyzuw', emb, w011) / nlat
    r101 = np.einsum('xyzb,buw->xyzuw', emb, w101) / nlat
    r110 = np.einsum('xyzb,buw->xyzuw', emb, w110) / nlat
    eye3 = np.eye(3)
    k00 = r000
    k01 = np.einsum('xyzuw,xyzk->xyzuwk', r011, sh1).reshape(5, 5, 5, MUL, 3 * MUL)
    k11 = np.einsum('xyzuw,ik->xyzuiwk', r101, eye3).reshape(5, 5, 5, 3 * MUL, 3 * MUL)
    k10 = np.einsum('xyzuw,xyzi->xyzuiw', r110, sh1).reshape(5, 5, 5, 3 * MUL, MUL) / np.sqrt(3.0)
    top = np.concatenate([k00, k01], axis=-1)
    bot = np.concatenate([k10, k11], axis=-1)
    k = np.concatenate([top, bot], axis=-2)

    lin00 = sc0 / np.sqrt(float(MUL))
    lin11 = np.einsum('uw,ik->uiwk', sc1 / np.sqrt(float(MUL)), eye3).reshape(3 * MUL, 3 * MUL)
    z16 = np.zeros((MUL, 3 * MUL))
    lin = np.concatenate([
        np.concatenate([lin00, z16], axis=1),
        np.concatenate([z16.T, lin11], axis=1)], axis=0)
    k[2, 2, 2] = lin
    return k


def _pack_weights(k):
    """[128, 75*128] with W[s*64+c, widx*128 + p*64+o] = k[tx,ty,zb+s-p,c,o]."""
    Ws = np.zeros((N_W, 128, 128))
    for tx in range(5):
        for ty in range(5):
            for zbi, zb in enumerate(ZB_LIST):
                w = Ws[(tx * 5 + ty) * 3 + zbi]
                for s in range(2):
                    for p in range(2):
                        tz = zb + s - p
                        if 0 <= tz <= 4:
                            w[s * 64:(s + 1) * 64, p * 64:(p + 1) * 64] = k[tx, ty, tz]
    return np.ascontiguousarray(
        Ws.transpose(1, 0, 2).reshape(128, N_W * 128)).astype(np.float32)


_NC = None


def _get_nc():
    global _NC
    if _NC is None:
        _NC = _build_nc()
    return _NC


def _build_nc():
    nc = bacc.Bacc("TRN2", target_bir_lowering=False)
    f32 = mybir.dt.float32
    f32r = mybir.dt.float32r

    xin = nc.dram_tensor("xin", [64, PX * PLANE], f32r, kind="ExternalInput")
    wts = nc.dram_tensor("wts", [128, N_W * 128], f32r, kind="ExternalInput")
    yout = nc.dram_tensor("yout", [64, OX * OPLANE], f32, kind="ExternalOutput")

    with TileContext(nc) as tc:
        with tc.tile_pool(name="wpool", bufs=1) as wpool, \
             tc.tile_pool(name="xpool", bufs=7) as xpool, \
             tc.tile_pool(name="opool", bufs=2) as opool, \
             tc.tile_pool(name="ppool", bufs=4, space="PSUM") as ppool:

            wt = wpool.tile([128, N_W * 128], f32r)
            nc.sync.dma_start(out=wt[:, :], in_=wts[:, :])

            planes = {}

            def get_plane(px):
                if px not in planes:
                    pt = xpool.tile([128, PLANE], f32r, tag="plane", name="plane")
                    base = px * PLANE
                    nc.sync.dma_start(out=pt[0:64, :],
                                      in_=xin[:, base:base + PLANE])
                    nc.sync.dma_start(out=pt[64:128, 0:PLANE - 1],
                                      in_=xin[:, base + 1:base + PLANE])
                    planes[px] = pt
                return planes[px]

            for xo in range(OX):
                ostage = opool.tile([64, OPLANE], f32, name="ostage")
                ostv = ostage.rearrange("c (y z) -> c y z", z=OZ)
                for yb in range(YB):
                    y0 = yb * YBS
                    ps = ppool.tile([128, YBS * (OZ // 2)], f32, name="ps")
                    i = 0
                    for tx in range(5):
                        pt = get_plane(xo + tx)
                        ptv = pt.rearrange("c (y z) -> c y z", z=PZ)
                        for ty in range(5):
                            for zbi, zb in enumerate(ZB_LIST):
                                rhs = ptv[:, y0 + ty:y0 + ty + YBS,
                                          zb:zb + OZ:2]
                                widx = (tx * 5 + ty) * 3 + zbi
                                lhsT = wt[:, widx * 128:(widx + 1) * 128]
                                nc.tensor.matmul(ps[:, :], lhsT, rhs,
                                                 start=(i == 0), stop=(i == N_W - 1))
                                i += 1
                    psv = ps.rearrange("c (y z) -> c y z", z=OZ // 2)
                    for p in range(2):
                        nc.vector.tensor_copy(ostv[:, y0:y0 + YBS, p:OZ:2],
                                              psv[p * 64:(p + 1) * 64, :, :])
                nc.sync.dma_start(out=yout[:, xo * OPLANE:(xo + 1) * OPLANE],
                                  in_=ostage[:, :])
    nc.finalize()
    return nc


def _prep_inputs(x, wts_arr):
    """Returns per-core in_maps. x: [2,48,48,48,64] float32."""
    in_maps = []
    for core in range(N_CORES):
        n, xs = core // 4, (core % 4) * OX
        xpadn = np.pad(x[n], ((2, 2), (2, 2), (2, 2), (0, 0)))
        slab = xpadn[xs:xs + PX]                               # [16,52,52,64]
        xc = np.ascontiguousarray(slab.transpose(3, 0, 1, 2))  # [64,16,52,52]
        in_maps.append({
            "xin": xc.reshape(64, PX * PLANE),
            "wts": wts_arr,
        })
    return in_maps


def _run(inputs, trace=False):
    x = np.asarray(inputs["x"], np.float32)
    k = _build_k(np.asarray(inputs["w000"], np.float64),
                 np.asarray(inputs["w011"], np.float64),
                 np.asarray(inputs["w101"], np.float64),
                 np.asarray(inputs["w110"], np.float64),
                 np.asarray(inputs["sc0"], np.float64),
                 np.asarray(inputs["sc1"], np.float64))
    wts_arr = _pack_weights(k)
    in_maps = _prep_inputs(x, wts_arr)

    nc = _get_nc()
    res = bass_utils.run_bass_kernel_spmd(
        nc, in_maps, core_ids=list(range(N_CORES)), trace=trace)

    out = np.empty((2, 48, 48, 48, 64), np.float32)
    for core in range(N_CORES):
        n, xs = core // 4, (core % 4) * OX
        oc = res.results[core]["yout"].reshape(64, OX, OY, OZ)
        out[n, xs:xs + OX] = oc.transpose(1, 2, 3, 0)
    return out, res


def kernel(**inputs):
    out, _ = _run(inputs, trace=False)
    return out


# revision 4
# speedup vs baseline: 1.8380x; 1.8380x over previous
"""Trainium2 Bass kernel for the e3nn-style 5x5x5 SAME conv (dense_cnn).

Strategy
--------
Data-parallel: 8 shards = 2 batches x 4 x-slabs of 12 output planes each.
Each core gets a zero/halo-padded, channel-first input slab [64, 16, 52, 52]
and produces [64, 12, 48, 48].

On-device the conv is a sum of 75 (=5x5x3) PSUM-accumulated matmuls per
output tile:
  - SBUF holds each input x-plane as a "dup" tile [128, 52*52]:
    partitions 0-63  = channel c at voxel v
    partitions 64-127= channel c at voxel v+1 (one z-voxel shift)
  - A matmul with K=128 therefore applies TWO z-taps at once, and the
    stationary weights [128, 128] map them to TWO output z-parities
    (out partitions = (parity p, channel o)). Covering tz in 0..4 takes
    3 matmuls (z-bases 0, 2, 4) per (tx, ty): 10 of 12 weight blocks
    useful -> 83% PE utilization ceiling.
  - Weights/moving data are fp32 bitcast to float32r (full-rate fp32
    matmul for moving dim >= 256; slight HW precision reduction).

The tiny 5x5x5x64x64 kernel build (radial basis x Clebsch-Gordan) is done
on the host in numpy and shipped as a packed [128, 75*128] weight input,
replicated to every core.
"""

import math

import numpy as np

import concourse.bass as bass
import concourse.mybir as mybir


def _np_mm_dtype():
    import ml_dtypes
    return {"float32r": np.float32, "float16": np.float16,
            "bfloat16": ml_dtypes.bfloat16}[MM_DTYPE]
from concourse import bacc, bass_utils
from concourse.tile import TileContext

MUL = 16
NB = 4
R = 2.5

N_CORES = 8
MM_DTYPE = "float16"             # matmul operand dtype: float32r|float16|bfloat16
PX, PY, PZ = 16, 52, 52          # padded per-core input slab (x, y, z)
OX, OY, OZ = 12, 48, 48          # per-core output region
PLANE = PY * PZ                  # 2704 voxels per x-plane
OPLANE = OY * OZ                 # 2304 outputs per x-plane
ZB_LIST = (0, 2, 4)              # z-base offsets; taps tz = zb + s - p
N_W = 5 * 5 * 3                  # 75 packed weight matrices
YB = 3                           # y-blocks of 16 rows -> N = 16*24 = 384
YBS = OY // YB


def _build_k(w000, w011, w101, w110, sc0, sc1):
    """Numpy port of the reference kernel build. Returns [5,5,5,64,64]."""
    s = 2
    c = np.arange(-s, s + 1.0)
    lat = np.stack(np.meshgrid(c, c, c, indexing='ij'), axis=-1)
    norm = np.linalg.norm(lat, axis=-1)
    safe = np.where(norm == 0.0, 1.0, norm)
    nvec = np.where(norm[..., None] > 0.0, lat / safe[..., None], 0.0)
    sh1 = np.sqrt(3.0) * nvec
    values = np.linspace(0.0, R, NB + 2)[1:-1]
    step = R / (NB + 1)
    d = (norm[..., None] - values) / step
    dd = np.clip(d, -1.0 + 1e-9, 1.0 - 1e-9)
    emb = np.where(np.abs(d) < 1.0,
                   1.14136 * np.e ** 2 * np.exp(-1.0 / (1.0 - dd ** 2)), 0.0)
    nlat = 125.0

    r000 = np.einsum('xyzb,buw->xyzuw', emb, w000) / nlat
    r011 = np.einsum('xyzb,buw->xyzuw', emb, w011) / nlat
    r101 = np.einsum('xyzb,buw->xyzuw', emb, w101) / nlat
    r110 = np.einsum('xyzb,buw->xyzuw', emb, w110) / nlat
    eye3 = np.eye(3)
    k00 = r000
    k01 = np.einsum('xyzuw,xyzk->xyzuwk', r011, sh1).reshape(5, 5, 5, MUL, 3 * MUL)
    k11 = np.einsum('xyzuw,ik->xyzuiwk', r101, eye3).reshape(5, 5, 5, 3 * MUL, 3 * MUL)
    k10 = np.einsum('xyzuw,xyzi->xyzuiw', r110, sh1).reshape(5, 5, 5, 3 * MUL, MUL) / np.sqrt(3.0)
    top = np.concatenate([k00, k01], axis=-1)
    bot = np.concatenate([k10, k11], axis=-1)
    k = np.concatenate([top, bot], axis=-2)

    lin00 = sc0 / np.sqrt(float(MUL))
    lin11 = np.einsum('uw,ik->uiwk', sc1 / np.sqrt(float(MUL)), eye3).reshape(3 * MUL, 3 * MUL)
    z16 = np.zeros((MUL, 3 * MUL))
    lin = np.concatenate([
        np.concatenate([lin00, z16], axis=1),
        np.concatenate([z16.T, lin11], axis=1)], axis=0)
    k[2, 2, 2] = lin
    return k


def _pack_weights(k):
    """[128, 75*128] with W[s*64+c, widx*128 + p*64+o] = k[tx,ty,zb+s-p,c,o]."""
    Ws = np.zeros((N_W, 128, 128))
    for tx in range(5):
        for ty in range(5):
            for zbi, zb in enumerate(ZB_LIST):
                w = Ws[(tx * 5 + ty) * 3 + zbi]
                for s in range(2):
                    for p in range(2):
                        tz = zb + s - p
                        if 0 <= tz <= 4:
                            w[s * 64:(s + 1) * 64, p * 64:(p + 1) * 64] = k[tx, ty, tz]
    return np.ascontiguousarray(
        Ws.transpose(1, 0, 2).reshape(128, N_W * 128)).astype(_np_mm_dtype())


_NC = None


def _get_nc():
    global _NC
    if _NC is None:
        _NC = _build_nc()
    return _NC


def _build_nc():
    nc = bacc.Bacc("TRN2", target_bir_lowering=False)
    f32 = mybir.dt.float32
    fmm = getattr(mybir.dt, MM_DTYPE)

    xin = nc.dram_tensor("xin", [64, PX * PLANE], fmm, kind="ExternalInput")
    wts = nc.dram_tensor("wts", [128, N_W * 128], fmm, kind="ExternalInput")
    yout = nc.dram_tensor("yout", [64, OX * OPLANE], f32, kind="ExternalOutput")

    with TileContext(nc) as tc:
        with tc.tile_pool(name="wpool", bufs=1) as wpool, \
             tc.tile_pool(name="xpool", bufs=7) as xpool, \
             tc.tile_pool(name="opool", bufs=2) as opool, \
             tc.tile_pool(name="ppool", bufs=4, space="PSUM") as ppool:

            wt = wpool.tile([128, N_W * 128], fmm)
            nc.sync.dma_start(out=wt[:, :], in_=wts[:, :])

            planes = {}

            def get_plane(px):
                if px not in planes:
                    pt = xpool.tile([128, PLANE], fmm, tag="plane", name="plane")
                    base = px * PLANE
                    nc.sync.dma_start(out=pt[0:64, :],
                                      in_=xin[:, base:base + PLANE])
                    nc.sync.dma_start(out=pt[64:128, 0:PLANE - 1],
                                      in_=xin[:, base + 1:base + PLANE])
                    planes[px] = pt
                return planes[px]

            for xo in range(OX):
                ostage = opool.tile([64, OPLANE], f32, name="ostage")
                ostv = ostage.rearrange("c (y z) -> c y z", z=OZ)
                for yb in range(YB):
                    y0 = yb * YBS
                    ps = ppool.tile([128, YBS * (OZ // 2)], f32, name="ps")
                    i = 0
                    for tx in range(5):
                        pt = get_plane(xo + tx)
                        ptv = pt.rearrange("c (y z) -> c y z", z=PZ)
                        for ty in range(5):
                            for zbi, zb in enumerate(ZB_LIST):
                                rhs = ptv[:, y0 + ty:y0 + ty + YBS,
                                          zb:zb + OZ:2]
                                widx = (tx * 5 + ty) * 3 + zbi
                                lhsT = wt[:, widx * 128:(widx + 1) * 128]
                                nc.tensor.matmul(ps[:, :], lhsT, rhs,
                                                 start=(i == 0), stop=(i == N_W - 1))
                                i += 1
                    psv = ps.rearrange("c (y z) -> c y z", z=OZ // 2)
                    for p in range(2):
                        nc.vector.tensor_copy(ostv[:, y0:y0 + YBS, p:OZ:2],
                                              psv[p * 64:(p + 1) * 64, :, :])
                nc.sync.dma_start(out=yout[:, xo * OPLANE:(xo + 1) * OPLANE],
                                  in_=ostage[:, :])
    nc.finalize()
    return nc


def _prep_inputs(x, wts_arr):
    """Returns per-core in_maps. x: [2,48,48,48,64] float32."""
    in_maps = []
    for core in range(N_CORES):
        n, xs = core // 4, (core % 4) * OX
        xpadn = np.pad(x[n], ((2, 2), (2, 2), (2, 2), (0, 0)))
        slab = xpadn[xs:xs + PX]                               # [16,52,52,64]
        xc = np.ascontiguousarray(
            slab.transpose(3, 0, 1, 2).astype(_np_mm_dtype()))  # [64,16,52,52]
        in_maps.append({
            "xin": xc.reshape(64, PX * PLANE),
            "wts": wts_arr,
        })
    return in_maps


def _run(inputs, trace=False):
    x = np.asarray(inputs["x"], np.float32)
    k = _build_k(np.asarray(inputs["w000"], np.float64),
                 np.asarray(inputs["w011"], np.float64),
                 np.asarray(inputs["w101"], np.float64),
                 np.asarray(inputs["w110"], np.float64),
                 np.asarray(inputs["sc0"], np.float64),
                 np.asarray(inputs["sc1"], np.float64))
    wts_arr = _pack_weights(k)
    in_maps = _prep_inputs(x, wts_arr)

    nc = _get_nc()
    res = bass_utils.run_bass_kernel_spmd(
        nc, in_maps, core_ids=list(range(N_CORES)), trace=trace)

    out = np.empty((2, 48, 48, 48, 64), np.float32)
    for core in range(N_CORES):
        n, xs = core // 4, (core % 4) * OX
        oc = res.results[core]["yout"].reshape(64, OX, OY, OZ)
        out[n, xs:xs + OX] = oc.transpose(1, 2, 3, 0)
    return out, res


def kernel(**inputs):
    out, _ = _run(inputs, trace=False)
    return out
